# revision 12
# baseline (speedup 1.0000x reference)
"""Stereo cost-volume kernel for Trainium2 (8 NeuronCores, SPMD).

cost[n, j, h, x] = sum_c L[n,c,h,x] * R[n,c,h,x-j], zero when x < j.

Per core (h-sharded 1/8 slab of every pyramid level), groups of 8 blocks:
  - PE Gram tiles, K=C=32, bf16 inputs, fp32 PSUM (bank-aligned pairs):
      G[m, n] = sum_c R[c, b+m] * L[c, b+n]   (j = n - m >= 0 on free axis)
  - PSUM -> SBUF cast-copies to bf16, split 96/32 partitions (DVE + ACT)
  - group shear-DMA (diagonal AP, T=8 blocks/op): s2[p, t, j] = G_t[p, p+j]
  - PE transpose (identity matmul, bf16) -> pt[j, x'] PSUM
  - copy to SBUF octet tile, merged DMA-out (1KB runs) into right-padded
    bf16 DRAM dev[j, h, x]; host upcasts, slices pad, zero-fills x < j.
Emission is software-pipelined at group granularity.
"""
import numpy as np

# (W, H, D, Hc) per pyramid level; Hc = per-core h-slab with 8 cores
LEVELS = [(512, 256, 192, 32), (256, 128, 96, 16), (128, 64, 48, 8)]
N_CORES = 8
C = 32
T = 8  # blocks per shear/out group


def _build_program():
    import concourse.bacc as bacc
    import concourse.mybir as mybir
    import concourse.bass as bass
    from concourse.tile import TileContext

    f32 = mybir.dt.float32
    bf16 = mybir.dt.bfloat16

    nc = bacc.Bacc("TRN2", target_bir_lowering=False, debug=False)

    ins = {}
    outs = {}
    for li, (W, H, D, Hc) in enumerate(LEVELS):
        ins[f"lp{li}"] = nc.dram_tensor(f"lp{li}", [C, Hc, W + D], f32, kind="ExternalInput")
        ins[f"r{li}"] = nc.dram_tensor(f"r{li}", [C, Hc, W], f32, kind="ExternalInput")
        Wp = W + D - 1
        outs[f"dev{li}"] = nc.dram_tensor(f"dev{li}", [D, Hc, Wp], bf16, kind="ExternalOutput")
    idm = nc.dram_tensor("idm", [128, 128], f32, kind="ExternalInput")

    with TileContext(nc) as tc:
        with tc.tile_pool(name="const", bufs=1) as cpool:
            identb = cpool.tile([128, 128], bf16)
            nc.gpsimd.dma_start(identb[:], idm[:])  # SWDGE cast f32->bf16

            for li, (W, H, D, Hc) in enumerate(LEVELS):
                NW = 128 + D
                NA = 96 + D   # ga columns (rows 0..95 read cols [p, p+D))
                NB = 128 + D - 96  # gb columns (rows 96..127 read cols [96, 128+D))
                Wp = W + D - 1
                lp, r, dev = ins[f"lp{li}"], ins[f"r{li}"], outs[f"dev{li}"]
                devt = dev.ap().tensor

                blocks = [(h, b) for h in range(Hc) for b in range(0, W, 128)]
                assert len(blocks) % T == 0
                groups = [blocks[i:i + T] for i in range(0, len(blocks), T)]
                bpw = W // 128  # blocks per h-row
                jchunks = []
                j0 = 0
                while j0 < D:
                    jchunks.append((j0, min(96, D - j0)))
                    j0 += 96
                nch = len(jchunks)

                with (
                    tc.tile_pool(name=f"slab{li}", bufs=1) as spool,
                    tc.tile_pool(name=f"work{li}", bufs=4) as wpool,
                    tc.tile_pool(name=f"gpsum{li}", bufs=2, space="PSUM") as gpool,
                    tc.tile_pool(name=f"tpsum{li}", bufs=4, space="PSUM") as tpool,
                ):
                    lt = spool.tile([C, Hc, W + D], bf16)
                    rt = spool.tile([C, Hc, W], bf16)
                    nc.gpsimd.dma_start(lt[:], lp[:])   # SWDGE cast f32 -> bf16
                    nc.gpsimd.dma_start(rt[:], r[:])

                    state = {}

                    def st_mm(gi):
                        # 4 psum pair-tiles; interleave mm + drain copies
                        ga = wpool.tile([96, T, NA], bf16, tag="ga")
                        gb = wpool.tile([32, T, NB], bf16, tag="gb")
                        state[(gi, "gab")] = (ga, gb)
                        for pi in range(T // 2):
                            g = gpool.tile([128, 1024], f32, tag="g")
                            for t in range(2):
                                h, b = groups[gi][2 * pi + t]
                                nc.tensor.matmul(
                                    g[:, 512 * t:512 * t + NW],
                                    rt[:, h, b:b + 128],
                                    lt[:, h, b:b + NW],
                                    start=True, stop=True,
                                )
                            # drain this psum pair into group tiles
                            gsa = bass.AP(g.tensor, g.offset,
                                          [[g.ap[0][0], 128], [512, 2], [1, NA]])
                            gsb = bass.AP(g.tensor, g.offset + 96,
                                          [[g.ap[0][0], 128], [512, 2], [1, NB]])
                            nc.vector.tensor_copy(
                                ga[:, 2 * pi:2 * pi + 2, :], gsa[0:96])
                            nc.scalar.copy(
                                gb[:, 2 * pi:2 * pi + 2, :], gsb[96:128])

                    def st_shear(gi):
                        sh_eng = nc.sync if gi % 2 == 0 else nc.scalar
                        ga, gb = state.pop((gi, "gab"))
                        s2 = wpool.tile([128, T, D], bf16, tag="s2")
                        # rows 0..95: col p + j  (ga holds cols [0, NA))
                        sh_eng.dma_start(
                            bass.AP(s2.tensor, s2.offset,
                                    [[T * D, 96], [D, T], [1, D]]),
                            bass.AP(ga.tensor, ga.offset,
                                    [[T * NA + 1, 96], [NA, T], [1, D]]),
                        )
                        # rows 96..127: col p + j; gb holds cols [96, 128 + D)
                        sh_eng.dma_start(
                            bass.AP(s2.tensor, s2.offset + 96 * T * D,
                                    [[T * D, 32], [D, T], [1, D]]),
                            bass.AP(gb.tensor, gb.offset,
                                    [[T * NB + 1, 32], [NB, T], [1, D]]),
                        )
                        state[(gi, "s2")] = s2

                    def st_tr(gi):
                        s2 = state.pop((gi, "s2"))
                        st = wpool.tile([96, nch, T, 128], bf16, tag="st")
                        state[(gi, "st")] = st
                        for half in range(2):
                            pt = tpool.tile([96, nch, 4, 128], bf16, tag="pt")
                            for t4 in range(4):
                                t = half * 4 + t4
                                for ci, (j0, jn) in enumerate(jchunks):
                                    nc.tensor.transpose(
                                        pt[0:jn, ci, t4, :],
                                        s2[:, t, j0:j0 + jn],
                                        identb[:],
                                    )
                            dstap = bass.AP(
                                st.tensor, st.offset + half * 4 * 128,
                                [[nch * T * 128, 96], [T * 128, nch], [128, 4], [1, 128]])
                            if half == 0:
                                nc.vector.tensor_copy(dstap, pt[:])
                            else:
                                nc.scalar.copy(dstap, pt[:])

                    def st_out(gi):
                        dma_eng = nc.scalar if gi % 2 == 0 else nc.sync
                        st = state.pop((gi, "st"))
                        grp = groups[gi]
                        (h0, b0) = grp[0]
                        nrows = T // bpw  # h-rows in this group
                        for ci, (j0, jn) in enumerate(jchunks):
                            o0 = (j0 * Hc + h0) * Wp + b0 + j0
                            srcap = bass.AP(st.tensor, st.offset + ci * T * 128,
                                            [[nch * T * 128, jn], [1, T * 128]])
                            if nrows == 1:
                                dst = bass.AP(devt, o0,
                                              [[Hc * Wp + 1, jn], [1, T * 128]])
                            else:
                                dst = bass.AP(devt, o0,
                                              [[Hc * Wp + 1, jn], [Wp, nrows],
                                               [1, (T // nrows) * 128]])
                            dma_eng.dma_start(dst, srcap)

                    stages = [st_mm, st_shear, st_tr, st_out]
                    ns = len(stages)
                    ng = len(groups)
                    for step in range(ng + ns - 1):
                        for s in range(ns):
                            i = step - s
                            if 0 <= i < ng:
                                stages[s](i)
    nc.compile()
    return nc


_PROGRAM_CACHE = {}


def _get_program():
    if "nc" not in _PROGRAM_CACHE:
        _PROGRAM_CACHE["nc"] = _build_program()
    return _PROGRAM_CACHE["nc"]


def kernel(left0, right0, left1, right1, left2, right2, _trace=False):
    from concourse.bass_utils import run_bass_kernel_spmd

    lefts = [left0, left1, left2]
    rights = [right0, right1, right2]
    nc = _get_program()

    idm = np.eye(128, dtype=np.float32)
    in_maps = []
    for k in range(N_CORES):
        m = {"idm": idm}
        for li, (W, H, D, Hc) in enumerate(LEVELS):
            Ls = np.asarray(lefts[li])[0, :, k * Hc:(k + 1) * Hc, :]
            Rs = np.asarray(rights[li])[0, :, k * Hc:(k + 1) * Hc, :]
            m[f"lp{li}"] = np.ascontiguousarray(
                np.pad(Ls, ((0, 0), (0, 0), (0, D))), dtype=np.float32)
            m[f"r{li}"] = np.ascontiguousarray(Rs, dtype=np.float32)
        in_maps.append(m)

    kw = {}
    if _trace:
        kw = dict(trace=True, trace_cores=[0])
    results = run_bass_kernel_spmd(nc, in_maps, core_ids=list(range(N_CORES)), **kw)

    out = []
    for li, (W, H, D, Hc) in enumerate(LEVELS):
        vol = np.zeros((1, D, H, W), dtype=np.float32)
        jj = np.arange(D)[:, None, None]
        xx = np.arange(W)[None, None, :]
        band = xx >= jj
        for k in range(N_CORES):
            dv = np.asarray(results.results[k][f"dev{li}"][:, :, :W], dtype=np.float32)
            vol[0, :, k * Hc:(k + 1) * Hc, :] = np.where(band, dv, 0.0)
        out.append(vol)
    if _trace:
        return tuple(out), results
    return tuple(out)


# revision 14
# speedup vs baseline: 1.1043x; 1.1043x over previous
"""Stereo cost-volume kernel for Trainium2 (8 NeuronCores, SPMD).

cost[n, j, h, x] = sum_c L[n,c,h,x] * R[n,c,h,x-j], zero when x < j.

Per core (h-sharded 1/8 slab of every pyramid level), groups of 8 blocks:
  - PE Gram tiles, K=C=32, bf16 inputs, fp32 PSUM (bank-aligned pairs):
      G[m, n] = sum_c R[c, b+m] * L[c, b+n]   (j = n - m >= 0 on free axis)
  - PSUM -> SBUF cast-copies to bf16, split 96/32 partitions (DVE + ACT)
  - group shear-DMA (diagonal AP, T=8 blocks/op): s2[p, t, j] = G_t[p, p+j]
  - PE transpose (identity matmul, bf16) -> pt[j, x'] PSUM
  - copy to SBUF octet tile, merged DMA-out (1KB runs) into right-padded
    bf16 DRAM dev[j, h, x]; host upcasts, slices pad, zero-fills x < j.
Emission is software-pipelined at group granularity.
"""
import numpy as np

# (W, H, D, Hc) per pyramid level; Hc = per-core h-slab with 8 cores
LEVELS = [(512, 256, 192, 32), (256, 128, 96, 16), (128, 64, 48, 8)]
N_CORES = 8
C = 32
T = 8  # blocks per shear/out group


def _build_program():
    import concourse.bacc as bacc
    import concourse.mybir as mybir
    import concourse.bass as bass
    from concourse.tile import TileContext

    f32 = mybir.dt.float32
    bf16 = mybir.dt.bfloat16

    nc = bacc.Bacc("TRN2", target_bir_lowering=False, debug=False)

    ins = {}
    outs = {}
    for li, (W, H, D, Hc) in enumerate(LEVELS):
        ins[f"lp{li}"] = nc.dram_tensor(f"lp{li}", [C, Hc, W + D], f32, kind="ExternalInput")
        ins[f"r{li}"] = nc.dram_tensor(f"r{li}", [C, Hc, W], f32, kind="ExternalInput")
        Wp = W + D - 1
        outs[f"dev{li}"] = nc.dram_tensor(f"dev{li}", [D, Hc, Wp], bf16, kind="ExternalOutput")
    idm = nc.dram_tensor("idm", [128, 128], f32, kind="ExternalInput")

    with TileContext(nc) as tc:
        with tc.tile_pool(name="const", bufs=1) as cpool:
            identb = cpool.tile([128, 128], bf16)
            nc.gpsimd.dma_start(identb[:], idm[:])  # SWDGE cast f32->bf16

            for li, (W, H, D, Hc) in enumerate(LEVELS):
                NW = 128 + D
                NA = 96 + D   # ga columns (rows 0..95 read cols [p, p+D))
                NB = 128 + D - 96  # gb columns (rows 96..127 read cols [96, 128+D))
                Wp = W + D - 1
                lp, r, dev = ins[f"lp{li}"], ins[f"r{li}"], outs[f"dev{li}"]
                devt = dev.ap().tensor

                blocks = [(h, b) for h in range(Hc) for b in range(0, W, 128)]
                assert len(blocks) % T == 0
                groups = [blocks[i:i + T] for i in range(0, len(blocks), T)]
                bpw = W // 128  # blocks per h-row
                jchunks = []
                j0 = 0
                while j0 < D:
                    jchunks.append((j0, min(96, D - j0)))
                    j0 += 96
                nch = len(jchunks)

                with (
                    tc.tile_pool(name=f"slab{li}", bufs=1) as spool,
                    tc.tile_pool(name=f"work{li}", bufs=4) as wpool,
                    tc.tile_pool(name=f"gpsum{li}", bufs=3, space="PSUM") as gpool,
                    tc.tile_pool(name=f"tpsum{li}", bufs=2, space="PSUM") as tpool,
                ):
                    lt = spool.tile([C, Hc, W + D], bf16)
                    rt = spool.tile([C, Hc, W], bf16)
                    nc.gpsimd.dma_start(lt[:], lp[:])   # SWDGE cast f32 -> bf16
                    nc.gpsimd.dma_start(rt[:], r[:])

                    state = {}

                    def st_mm(gi):
                        # 4 psum pair-tiles; interleave mm + drain copies
                        ga = wpool.tile([96, T, NA], bf16, tag="ga")
                        gb = wpool.tile([32, T, NB], bf16, tag="gb")
                        state[(gi, "gab")] = (ga, gb)
                        for pi in range(T // 2):
                            g = gpool.tile([128, 1024], f32, tag="g")
                            for t in range(2):
                                h, b = groups[gi][2 * pi + t]
                                nc.tensor.matmul(
                                    g[:, 512 * t:512 * t + NW],
                                    rt[:, h, b:b + 128],
                                    lt[:, h, b:b + NW],
                                    start=True, stop=True,
                                )
                            # drain this psum pair into group tiles
                            gsa = bass.AP(g.tensor, g.offset,
                                          [[g.ap[0][0], 128], [512, 2], [1, NA]])
                            gsb = bass.AP(g.tensor, g.offset + 96,
                                          [[g.ap[0][0], 128], [512, 2], [1, NB]])
                            nc.scalar.copy(
                                ga[:, 2 * pi:2 * pi + 2, :], gsa[0:96])
                            nc.vector.tensor_copy(
                                gb[:, 2 * pi:2 * pi + 2, :], gsb[96:128])

                    def st_shear(gi):
                        ga, gb = state.pop((gi, "gab"))
                        s2 = wpool.tile([128, T, D], bf16, tag="s2")
                        # rows 0..95: col p + j  (ga holds cols [0, NA))
                        nc.sync.dma_start(
                            bass.AP(s2.tensor, s2.offset,
                                    [[T * D, 96], [D, T], [1, D]]),
                            bass.AP(ga.tensor, ga.offset,
                                    [[T * NA + 1, 96], [NA, T], [1, D]]),
                        )
                        # rows 96..127: col p + j; gb holds cols [96, 128 + D)
                        nc.sync.dma_start(
                            bass.AP(s2.tensor, s2.offset + 96 * T * D,
                                    [[T * D, 32], [D, T], [1, D]]),
                            bass.AP(gb.tensor, gb.offset,
                                    [[T * NB + 1, 32], [NB, T], [1, D]]),
                        )
                        state[(gi, "s2")] = s2

                    def st_tr(gi):
                        s2 = state.pop((gi, "s2"))
                        st = wpool.tile([96, nch, T, 128], bf16, tag="st")
                        state[(gi, "st")] = st
                        for half in range(2):
                            pt = tpool.tile([96, nch, 4, 128], bf16, tag="pt")
                            for t4 in range(4):
                                t = half * 4 + t4
                                for ci, (j0, jn) in enumerate(jchunks):
                                    nc.tensor.transpose(
                                        pt[0:jn, ci, t4, :],
                                        s2[:, t, j0:j0 + jn],
                                        identb[:],
                                    )
                            dstap = bass.AP(
                                st.tensor, st.offset + half * 4 * 128,
                                [[nch * T * 128, 96], [T * 128, nch], [128, 4], [1, 128]])
                            if half == 0:
                                nc.vector.tensor_copy(dstap, pt[:])
                            else:
                                nc.scalar.copy(dstap, pt[:])

                    def st_out(gi):
                        dma_eng = nc.scalar if gi % 2 == 0 else nc.sync
                        st = state.pop((gi, "st"))
                        grp = groups[gi]
                        (h0, b0) = grp[0]
                        nrows = T // bpw  # h-rows in this group
                        for ci, (j0, jn) in enumerate(jchunks):
                            o0 = (j0 * Hc + h0) * Wp + b0 + j0
                            srcap = bass.AP(st.tensor, st.offset + ci * T * 128,
                                            [[nch * T * 128, jn], [1, T * 128]])
                            if nrows == 1:
                                dst = bass.AP(devt, o0,
                                              [[Hc * Wp + 1, jn], [1, T * 128]])
                            else:
                                dst = bass.AP(devt, o0,
                                              [[Hc * Wp + 1, jn], [Wp, nrows],
                                               [1, (T // nrows) * 128]])
                            dma_eng.dma_start(dst, srcap)

                    stages = [st_mm, st_shear, st_tr, st_out]
                    ns = len(stages)
                    ng = len(groups)
                    for step in range(ng + ns - 1):
                        for s in range(ns):
                            i = step - s
                            if 0 <= i < ng:
                                stages[s](i)
    nc.compile()
    return nc


_PROGRAM_CACHE = {}


def _get_program():
    if "nc" not in _PROGRAM_CACHE:
        _PROGRAM_CACHE["nc"] = _build_program()
    return _PROGRAM_CACHE["nc"]


def kernel(left0, right0, left1, right1, left2, right2, _trace=False):
    from concourse.bass_utils import run_bass_kernel_spmd

    lefts = [left0, left1, left2]
    rights = [right0, right1, right2]
    nc = _get_program()

    idm = np.eye(128, dtype=np.float32)
    in_maps = []
    for k in range(N_CORES):
        m = {"idm": idm}
        for li, (W, H, D, Hc) in enumerate(LEVELS):
            Ls = np.asarray(lefts[li])[0, :, k * Hc:(k + 1) * Hc, :]
            Rs = np.asarray(rights[li])[0, :, k * Hc:(k + 1) * Hc, :]
            m[f"lp{li}"] = np.ascontiguousarray(
                np.pad(Ls, ((0, 0), (0, 0), (0, D))), dtype=np.float32)
            m[f"r{li}"] = np.ascontiguousarray(Rs, dtype=np.float32)
        in_maps.append(m)

    kw = {}
    if _trace:
        kw = dict(trace=True, trace_cores=[0])
    results = run_bass_kernel_spmd(nc, in_maps, core_ids=list(range(N_CORES)), **kw)

    out = []
    for li, (W, H, D, Hc) in enumerate(LEVELS):
        vol = np.zeros((1, D, H, W), dtype=np.float32)
        jj = np.arange(D)[:, None, None]
        xx = np.arange(W)[None, None, :]
        band = xx >= jj
        for k in range(N_CORES):
            dv = np.asarray(results.results[k][f"dev{li}"][:, :, :W], dtype=np.float32)
            vol[0, :, k * Hc:(k + 1) * Hc, :] = np.where(band, dv, 0.0)
        out.append(vol)
    if _trace:
        return tuple(out), results
    return tuple(out)
